# revision 34
# baseline (speedup 1.0000x reference)
"""Trainium2 Bass kernel for madmom-style DBN downbeat tracking (Viterbi decode).

Contract: kernel(**inputs) -> (path int32 (F,), logp float32), matching
reference.reference(). Self-contained: all shapes/constants hardcoded or
derived from the passed input arrays.

Algorithm (validated against the reference in numpy, exact path match):
The Viterbi recurrence over S~14.9k states collapses to a recurrence over the
240 beat-boundary ("first") states f_t[b,i] (4 beats x 60 tempo intervals):

    f_t[b,i] = max_j( F[b-1, j, t-iv[j]] + Sc(b,j,t) + T[j,i] )

where iv[j] in [28,109] are the tempo interval lengths, Sc is a
host-precomputable prefix-sum observation term (with the per-frame obs
constant folded in), and T = log transition matrix (banded: j-i in [-15,21]).
Since min(iv)=28, frames are processed in blocks of 28 (one device step per
block, 215 blocks, strictly sequential).

Device layout per block (partition dim = (beat,tau) = 112 rows):
  T1  (PE)  4 matmuls: gather delayed F values from the slot-indexed skewed
            SBUF history (FHS[d, b, c], c = t + d + 28, d = iv-28 in [0,82))
            via a 0/1 selector rhs that also compacts slots -> 60 columns.
  V1  (DVE) add Sc block               [112, 60]
  V2  (DVE) banded build tmp = LamPad[window i+r] + Tband   [112, 60*37]
  V3  (DVE) window max-reduce -> F_k   [112, 60]
  T2  (PE)  transpose F_k -> [60, 112]
  C2b (ACT) copy rows 0..27 (slot==j identity there) -> staging
  D1b (DMA) diagonal SBUF->SBUF write rows 0..27 into FHS   [critical]
  T3  (PE)  slot-pad rows 28..81 via selector matmul; C3 copy; D1z diag DMA
            [1-block slack, emitted after next block's T1]
  DF  (DMA) F_k -> DRAM output (all f values; host uses them for backtrack)

Host: precompute (f64 prefix sums) Sc blocks / band T / virtual (t<0) history;
afterwards compute final v over all states, argmax/logp, and backtrack,
recomputing boundary argmaxes bit-identically to the device build.
"""

import math

import numpy as np

# ---------------------------------------------------------------- constants
F = 6000
NB = 4            # beats
TAU = 28          # frames per block (= min interval)
OBS_LAMBDA = 16.0
NEG = np.float32(-1.0e30)
NBLK = (F + TAU - 1) // TAU            # 215 (6000 % 28 != 0 -> 214*28=5992..)
assert NBLK == 215

# band of finite log_trans entries: j - i in [BLO, BHI]  (verified at runtime)
BLO, BHI = -15, 21
W = BHI - BLO + 1                      # 37
NI = 60                                # tempo intervals
BSTR = TAU                             # beat stride (rows = b*28 + tau)
P = NB * BSTR                          # 112 partitions
PADW = NI + W - 1                      # 96 (LamPad columns)
NSLOT = 82                             # delays 28..109 -> slots 0..81
CW = 6144                              # FHS per-beat column pitch
FHW = NB * CW                          # FHS free width per partition
NSHORT = 28                            # slots 0..27 have slot == j (iv=28+j)
PREC = 141                             # prefill cols (t<0 region, skew+30)

_cache = {}


# ------------------------------------------------------------ host precompute
def _statics(first_states, log_trans):
    """Derive iv, slot map, band tables from the inputs."""
    fs = np.asarray(first_states)
    nsb_first = fs[0]                                    # (60,)
    iv = np.diff(np.concatenate([nsb_first, [fs[1, 0]]])).astype(np.int64)
    T = np.asarray(log_trans)[0].astype(np.float32)      # (60, 60) from->to
    # verify band
    fin = T > -1.0e29
    for j in range(NI):
        idx = np.where(fin[:, j])[0] if False else None
    ji, ii = np.nonzero(fin)
    assert (ji - ii).min() >= BLO and (ji - ii).max() <= BHI, \
        (int((ji - ii).min()), int((ji - ii).max()))
    assert iv.min() == 28 and iv.max() == 109 and len(iv) == NI
    slot = (iv - 28).astype(np.int64)                    # (60,) strictly incr.
    assert np.all(slot[:NSHORT] == np.arange(NSHORT))    # identity short group
    return iv, slot, T


def _precompute(activations, iv, slot, T, num_states):
    """All host-side arrays (f64 prefix sums -> f32)."""
    acts = np.asarray(activations, np.float64)
    a_b, a_d = acts[:, 0], acts[:, 1]
    dens = np.stack([
        np.log((1.0 - a_b - a_d) / (OBS_LAMBDA - 1.0)),
        np.log(a_b),
        np.log(a_d),
    ], axis=1)                                           # (F,3) f64
    Pfx = np.concatenate([np.zeros((1, 3)), np.cumsum(dens, axis=0)])  # (F+1,3)

    def pref(c, t):
        t = np.clip(t, -1, F - 1)
        return Pfx[t + 1, c]

    m = np.ceil(iv / OBS_LAMBDA).astype(np.int64) - 1    # (60,)

    def S_of(b, t):
        """S(b, j, t) for all j, t vector; returns (len(t), 60)."""
        beta = 2 if b == 0 else 1
        t = np.asarray(t)[:, None]
        lo = t - iv[None, :] + 1
        return (pref(beta, lo + m[None, :]) - pref(beta, lo)
                + pref(0, t) - pref(0, lo + m[None, :]))

    # Sc blocks: Sc[k, b*TAU+tau, j] = S((b-1)%4, j, t-1) + dens[t, beta(b)]
    t_all = np.arange(F)
    Sc = np.zeros((NBLK, P, NI), np.float32)
    for b in range(NB):
        bp = (b - 1) % NB
        beta = 2 if b == 0 else 1
        block = S_of(bp, t_all - 1) + dens[t_all, beta][:, None]   # (F, 60)
        padn = NBLK * TAU - F
        blockp = np.concatenate([block, np.full((padn, NI), 0.0)])
        Sc[:, b * BSTR:b * BSTR + TAU, :] = (
            blockp.reshape(NBLK, TAU, NI).astype(np.float32))
    # frames >= F (tail of last block): keep candidates finite but dead.
    for t in range(F, NBLK * TAU):
        k, tau = divmod(t, TAU)
        for b in range(NB):
            Sc[k, b * BSTR + tau, :] = -0.5e30

    # virtual history values for t' < 0
    logS0 = np.float64(-np.log(np.float32(num_states)))
    maxiv = int(iv.max())
    fvirt = np.full((NB, NI, maxiv), 0.0)                # index [-t'] -> t'+maxiv
    for b in range(NB):
        beta = 2 if b == 0 else 1
        for j in range(NI):
            L = int(iv[j])
            thr = math.ceil(L / OBS_LAMBDA)
            for tp in range(-L, 0):                      # only t' >= -iv[j] used
                u = tp + L - 1
                if u == -1:
                    val = logS0
                else:
                    ks = np.arange(L - 1 - u, L)
                    classes = np.where(ks < thr, beta, 0)
                    xs = np.arange(0, u + 1)
                    val = logS0 + dens[xs, classes].sum()
                    lo = u - L + 1
                    val -= (pref(beta, np.array([lo + m[j]]))[0]
                            - pref(beta, np.array([lo]))[0]
                            + pref(0, np.array([u]))[0]
                            - pref(0, np.array([lo + m[j]]))[0])
                fvirt[b, j, tp + maxiv] = val

    # prefill: full initial DRAM history image
    pre = np.full((NSLOT, NB, CW), NEG, np.float32)
    slot_of = {int(s): j for j, s in enumerate(slot)}
    for d in range(NSLOT):
        j = slot_of.get(d)
        if j is None:
            continue
        L = int(iv[j])
        for c in range(PREC):
            tp = c - 30 - d
            if tp >= 0:
                continue                                  # overwritten in time
            if tp < -L:
                continue                                  # never read
            for b in range(NB):
                pre[d, b, c] = np.float32(fvirt[b, j, tp + maxiv])

    # band table (content same for every partition row)
    tband_row = np.full((NI, W), NEG, np.float32)
    for i in range(NI):
        for r in range(W):
            j = i + r + BLO
            if 0 <= j < NI:
                tband_row[i, r] = T[j, i]
    tband = np.broadcast_to(tband_row.reshape(1, NI * W), (P, NI * W)).copy()

    # selector matrices
    csel = np.zeros((NSLOT, NI), np.float32)
    csel[slot, np.arange(NI)] = 1.0
    pslot = np.zeros((32, NSLOT), np.float32)
    jj = np.arange(NSHORT, NI)
    pslot[jj - NSHORT, slot[jj]] = 1.0
    ident = np.eye(P, dtype=np.float32)

    return dens, Sc, fvirt, maxiv, pre, tband, csel, pslot, ident


# ------------------------------------------------------------ numpy reference
def _forward_numpy(Sc, pre_fvirt, maxiv, iv, T):
    """Bit-faithful numpy replay of the device block loop. Returns fh
    (F+maxiv, NB, NI) f32 with fh[t+maxiv] = f_t (t<0 virtual)."""
    fvirt, = pre_fvirt,
    fh = np.empty((F + maxiv + TAU, NB, NI), np.float32)
    fh[:maxiv] = np.moveaxis(fvirt, 2, 0).astype(np.float32)
    Tf = T.astype(np.float32)
    # banded build tables
    for k in range(NBLK):
        t0 = k * TAU
        # lamraw[(b,tau), j] = fh[t0+tau-iv[j], b-1, j]
        taus = np.arange(TAU)
        lam = np.empty((NB, TAU, NI), np.float32)
        for b in range(NB):
            bp = (b - 1) % NB
            tt = t0 + taus[:, None] - iv[None, :] + maxiv    # (TAU, NI)
            lam[b] = fh[tt, bp, np.arange(NI)[None, :]]
        lam = lam.reshape(NB * TAU, NI)
        Sck = Sc[k].reshape(NB, BSTR, NI)[:, :TAU].reshape(NB * TAU, NI)
        lamc = (lam + Sck).astype(np.float32)                # V1
        NPP = NB * TAU
        lampad = np.full((NPP, PADW), NEG, np.float32)
        lampad[:, -BLO:-BLO + NI] = lamc
        win = lampad[:, np.add.outer(np.arange(NI), np.arange(W))]
        tmp = (win + np.broadcast_to(
            _forward_numpy.tband_row, (NPP, NI, W))).astype(np.float32)  # V2
        fk = tmp.max(axis=2)                                  # V3
        fk3 = fk.reshape(NB, TAU, NI)
        tlim = min(TAU, F - t0)
        fh[t0 + maxiv: t0 + maxiv + tlim] = np.moveaxis(fk3, 1, 0)[:tlim]
    return fh[:F + maxiv]


# ---------------------------------------------------------------- bass kernel
def _build_bass(statics_key, Sc, pre_full, tband, csel, pslot, ident):
    import concourse.mybir as mybir
    from concourse import bacc
    from concourse.tile import TileContext
    from concourse.tile import add_dep_helper

    nc = bacc.Bacc(None, target_bir_lowering=False)
    dt = mybir.dt.float32
    sc_in = nc.dram_tensor("sc", [NBLK, P, NI], dt, kind="ExternalInput")
    fhsinit = nc.dram_tensor("fhsinit", [NSLOT, NB, CW], dt,
                             kind="ExternalInput")
    tb_in = nc.dram_tensor("tb", [P, NI * W], dt, kind="ExternalInput")
    cs_in = nc.dram_tensor("cs", [NSLOT, NI], dt, kind="ExternalInput")
    ps_in = nc.dram_tensor("ps", [32, NSLOT], dt, kind="ExternalInput")
    id_in = nc.dram_tensor("id", [P, P], dt, kind="ExternalInput")
    fout = nc.dram_tensor("fout", [NBLK, P, NI], dt, kind="ExternalOutput")
    fhsd = nc.dram_tensor("fhsd", [NSLOT, NB, CW], dt)
    NBCW = NB * CW
    NLONG = NSLOT - NSHORT
    NIS = NSHORT
    NIL = NI - NSHORT

    def fhs_diag(row0, nrows, t0):
        ap = fhsd[:, :, :].copy()
        ap.ap = mybir.VecI64Pair([[NBCW + 1, nrows], [CW, NB], [1, TAU]])
        ap.offset = row0 * NBCW + (t0 + 30 + row0)
        return ap

    def win_load(dma, wtile, row0, nrows, t0, dma2=None):
        """Load fhsd rows [row0, row0+nrows) cols [t0, t0+28) into wtile with
        beat-rotated columns: wtile[d, ((bp+1)%4)*28 + tau] = fhsd[row0+d, bp, t0+tau]."""
        src_a = fhsd[:, :, :].copy()
        src_a.ap = mybir.VecI64Pair([[NBCW, nrows], [CW, 3], [1, TAU]])
        src_a.offset = row0 * NBCW + t0 + 2
        dst_a = wtile[:, :].copy()
        dst_a.ap = mybir.VecI64Pair([[NB * TAU, nrows], [TAU, 3], [1, TAU]])
        dst_a.offset = TAU
        i1 = dma(dst_a, src_a)
        src_b = fhsd[:, :, :].copy()
        src_b.ap = mybir.VecI64Pair([[NBCW, nrows], [1, TAU]])
        src_b.offset = row0 * NBCW + 3 * CW + t0 + 2
        i2 = dma2(wtile[:, 0:TAU], src_b) if dma2 is not None else \
            dma(wtile[:, 0:TAU], src_b)
        return i1, i2

    def stg_src(tile, row0, nrows):
        ap = tile[:, :].copy()
        ap.ap = mybir.VecI64Pair([[P, nrows], [TAU, NB], [1, TAU]])
        ap.offset = row0 * P
        return ap

    with TileContext(nc) as tc:
        with (
            tc.tile_pool(name="const", bufs=1) as cpool,
            tc.tile_pool(name="sb", bufs=4) as pool,
            tc.tile_pool(name="ps", bufs=1, space="PSUM") as psp,
        ):
            lampad = cpool.tile([P, PADW], dt, tag="lampad")
            tbandt = cpool.tile([P, NI * W], dt, tag="tband")
            tmp = cpool.tile([P, NI * W], dt, tag="tmp")
            cselS = cpool.tile([NSHORT, NI], dt, tag="cselS")
            cselL = cpool.tile([NLONG, NI], dt, tag="cselL")
            pslott = cpool.tile([32, NSLOT], dt, tag="pslot")
            identt = cpool.tile([P, P], dt, tag="ident")
            # ---- init
            nc.vector.memset(lampad[:, :], float(NEG))
            nc.sync.dma_start(fhsd[:, :, :], fhsinit[:, :, :])
            nc.sync.dma_start(tbandt[:, :], tb_in[:, :])
            nc.sync.dma_start(cselS[:, :], cs_in[0:NSHORT, :])
            nc.sync.dma_start(cselL[:, :], cs_in[NSHORT:NSLOT, :])
            nc.sync.dma_start(pslott[:, :], ps_in[:, :])
            nc.sync.dma_start(identt[:, :], id_in[:, :])
            # PE warm-ups: absorb const-DMA waits
            dws = psp.tile([P, P], dt, tag="dwarm")
            nc.tensor.matmul(dws[0:NI, 0:NI], cselS[:, :], cselS[:, :],
                             start=True, stop=True, skip_group_check=True)
            nc.tensor.matmul(dws[0:NI, 0:NI], cselL[:, :], cselL[:, :],
                             start=True, stop=True, skip_group_check=True)
            nc.tensor.matmul(dws[0:NSLOT, 0:NSLOT], pslott[:, :],
                             pslott[:, :], start=True, stop=True,
                             skip_group_check=True)
            nc.tensor.matmul(dws[:, :], identt[:, :], identt[:, :],
                             start=True, stop=True, skip_group_check=True)

            lazy = None
            for k in range(NBLK):
                t0 = k * TAU
                # ---- window loads: long (SWDGE, 2-block slack) then short
                winL = pool.tile([NLONG, NB * TAU], dt, tag="winL")
                win_load(nc.gpsimd.dma_start, winL, NSHORT, NLONG, t0)
                winS = pool.tile([NSHORT, NB * TAU], dt, tag="winS")
                win_load(nc.sync.dma_start, winS, 0, NSHORT, t0,
                         dma2=nc.scalar.dma_start)
                # ---- T1: two matmuls accumulate -> one PSUM [112, 60]
                pst = psp.tile([P, NI], dt, tag="lamraw")
                nc.tensor.matmul(pst[:, :], winL[:, :], cselL[:, :],
                                 start=True, stop=False)
                nc.tensor.matmul(pst[:, :], winS[:, :], cselS[:, :],
                                 start=False, stop=True)
                # ---- deferred lazy history write from previous block
                if lazy is not None:
                    lz_tile, lz_t0 = lazy
                    nc.gpsimd.dma_start(
                        fhs_diag(NSHORT, NLONG, lz_t0),
                        stg_src(lz_tile, NSHORT, NLONG))
                    lazy = None
                # ---- V1: single Sc add
                sct = pool.tile([P, NI], dt, tag="sc")
                nc.scalar.dma_start(sct[:, :], sc_in[k, :, :])
                nc.vector.tensor_add(out=lampad[:, -BLO:-BLO + NI],
                                     in0=pst[:, :], in1=sct[:, :])
                # ---- V2a/V3a: short output columns i < 28
                lam_win_a = lampad[:, :].copy()
                lam_win_a.ap = mybir.VecI64Pair([[PADW, P], [1, NIS], [1, W]])
                lam_win_a.offset = 0
                nc.vector.tensor_add(out=tmp[:, 0:NIS * W], in0=lam_win_a,
                                     in1=tbandt[:, 0:NIS * W])
                tmp3a = tmp[:, :].copy()
                tmp3a.ap = mybir.VecI64Pair([[NI * W, P], [W, NIS], [1, W]])
                fk = pool.tile([P, NI], dt, tag="fk")
                i_v3a = nc.vector.tensor_reduce(out=fk[:, 0:NIS], in_=tmp3a,
                                                axis=mybir.AxisListType.X,
                                                op=mybir.AluOpType.max)
                # ---- short tail: transpose, copy, critical history write
                pst2a = psp.tile([NIS, P], dt, tag="ftra")
                nc.tensor.transpose(pst2a[:, :], fk[:, 0:NIS], identt[:, :])
                s60a = pool.tile([NIS, P], dt, tag="s60a")
                nc.scalar.copy(out=s60a[:, :], in_=pst2a[:, :])
                nc.sync.dma_start(fhs_diag(0, NSHORT, t0),
                                  stg_src(s60a, 0, NSHORT))
                # ---- V2b/V3b: long outputs (overlap the DRAM bounce)
                lam_win_b = lampad[:, :].copy()
                lam_win_b.ap = mybir.VecI64Pair([[PADW, P], [1, NIL], [1, W]])
                lam_win_b.offset = NIS
                i_v2b = nc.vector.tensor_add(out=tmp[:, NIS * W:NI * W],
                                             in0=lam_win_b,
                                             in1=tbandt[:, NIS * W:NI * W])
                add_dep_helper(i_v2b.ins, i_v3a.ins, sync=False,
                               reason="short tail first")
                tmp3b = tmp[:, :].copy()
                tmp3b.ap = mybir.VecI64Pair([[NI * W, P], [W, NIL], [1, W]])
                tmp3b.offset = NIS * W
                nc.vector.tensor_reduce(out=fk[:, NIS:NI], in_=tmp3b,
                                        axis=mybir.AxisListType.X,
                                        op=mybir.AluOpType.max)
                nc.gpsimd.dma_start(fout[k, :, :], fk[:, :])
                # ---- lazy path: transpose long cols, slot-pad, stage
                pst2b = psp.tile([32, P], dt, tag="ftrb")
                nc.tensor.transpose(pst2b[:, :], fk[:, NIS:NI], identt[:, :])
                s60b = pool.tile([32, P], dt, tag="s60b")
                nc.scalar.copy(out=s60b[:, :], in_=pst2b[:, :])
                pst3 = psp.tile([NSLOT, P], dt, tag="slotp")
                nc.tensor.matmul(pst3[:, :], pslott[:, :], s60b[:, :],
                                 start=True, stop=True)
                stg = pool.tile([NSLOT, P], dt, tag="stg")
                nc.scalar.copy(out=stg[:, :], in_=pst3[:, :])
                lazy = (stg, t0)
    nc.compile()
    return nc


TAU14 = 14
NBLK14 = (F + TAU14 - 1) // TAU14          # 429
H14 = 2
BST14 = 16                                  # per-beat row stride (14 + 2 pad)
P14 = H14 * NB * BST14                      # 128 rows: p = h*64 + b*16 + tau
NIH = 30                                    # outputs per h


def _dev14_arrays(dens, Sc, tband, iv, T):
    """Device arrays for the TAU14/H2 layout, derived from the 28-based
    host tables (Sc is [NBLK, P, NI] with rows b*28+tau)."""
    # flat S+c table per (b, t): Sval[b, t, j]
    Sval = np.zeros((NB, NBLK14 * TAU14, NI), np.float32)
    for b in range(NB):
        for k in range(NBLK):
            for tau in range(TAU):
                t = k * TAU + tau
                if t < NBLK14 * TAU14:
                    Sval[b, t] = Sc[k, b * TAU + tau]
    # frames >= F already poisoned in Sc (-0.5e30) for t < NBLK*TAU;
    # frames in [NBLK*TAU, NBLK14*TAU14) don't exist (6020 > 6006) - none.
    Sc14 = np.zeros((NBLK14, P14, NI), np.float32)
    for k in range(NBLK14):
        for h in range(H14):
            for b in range(NB):
                for tau in range(TAU14):
                    t = k * TAU14 + tau
                    row = h * 64 + b * BST14 + 2 + tau
                    Sc14[k, row] = Sval[b, t] if t < Sval.shape[1] else -0.5e30
    # band table rows: h0 -> outputs i' (0..29), h1 -> outputs 30+i'
    tb = np.full((P14, NIH * W), NEG, np.float32)
    trow = np.asarray(tband[0]).reshape(NI, W)
    for h in range(H14):
        rows = slice(h * 64, h * 64 + 64)
        tb[rows, :] = np.tile(trow[h * NIH:(h + 1) * NIH].reshape(1, NIH * W),
                              (64, 1))
    ident64 = np.tile(np.eye(64, dtype=np.float32), (2, 1))
    return Sc14, tb, ident64


def _build_bass14(Sc14, tb14, csel, pslot, ident64):
    import concourse.mybir as mybir
    from concourse import bacc
    from concourse.tile import TileContext
    from concourse.tile import add_dep_helper

    nc = bacc.Bacc(None, target_bir_lowering=False)
    dt = mybir.dt.float32
    sc_in = nc.dram_tensor("sc", [NBLK14, P14, NI], dt, kind="ExternalInput")
    fhsinit = nc.dram_tensor("fhsinit", [NSLOT, NB, CW], dt,
                             kind="ExternalInput")
    tb_in = nc.dram_tensor("tb", [P14, NIH * W], dt, kind="ExternalInput")
    cs_in = nc.dram_tensor("cs", [NSLOT, NI], dt, kind="ExternalInput")
    ps_in = nc.dram_tensor("ps", [32, NSLOT], dt, kind="ExternalInput")
    id_in = nc.dram_tensor("id", [P14, 64], dt, kind="ExternalInput")
    fout = nc.dram_tensor("fout", [NBLK14, P14, NIH], dt,
                          kind="ExternalOutput")
    fhsd = nc.dram_tensor("fhsd", [NSLOT, NB, CW], dt)
    NBCW = NB * CW
    NLONG = NSLOT - NSHORT
    LPW = 66                     # lampad columns

    # history skew: FHS[d, b, c] = F[b, j(d), c - 30 - d]
    def fhs_diag(row0, nrows, t0):
        ap = fhsd[:, :, :].copy()
        ap.ap = mybir.VecI64Pair([[NBCW + 1, nrows], [CW, NB], [1, TAU14]])
        ap.offset = row0 * NBCW + (t0 + 30 + row0)
        return ap

    def win_load(dma_a, dma_b, wtile, row0, nrows, t0):
        """win col map: col = bp*16 + 2 + tau holds F[bp, j(d), t0+tau];
        cols bp*16+{0,1} are 2 leading pad cols fed from old history
        (t0-2, t0-1) so no RAW on the previous block's write."""
        src_a = fhsd[:, :, :].copy()
        src_a.ap = mybir.VecI64Pair([[NBCW, nrows], [CW, 3], [1, 16]])
        src_a.offset = row0 * NBCW + t0
        dst_a = wtile[:, :].copy()
        dst_a.ap = mybir.VecI64Pair([[64, nrows], [1, 48]])
        dst_a.offset = 16
        dma_a(dst_a, src_a)
        src_b = fhsd[:, :, :].copy()
        src_b.ap = mybir.VecI64Pair([[NBCW, nrows], [1, 16]])
        src_b.offset = row0 * NBCW + 3 * CW + t0
        dma_b(wtile[:, 0:16], src_b)

    def stg_src(tile, row0, nrows):
        ap = tile[:, :].copy()
        ap.ap = mybir.VecI64Pair([[64, nrows], [BST14, NB], [1, TAU14]])
        ap.offset = row0 * 64 + 2
        return ap

    with TileContext(nc) as tc:
        with (
            tc.tile_pool(name="const", bufs=1) as cpool,
            tc.tile_pool(name="sb", bufs=6) as pool,
            tc.tile_pool(name="ps", bufs=1, space="PSUM") as psp,
        ):
            lampad = cpool.tile([P14, LPW], dt, tag="lampad")
            tbandt = cpool.tile([P14, NIH * W], dt, tag="tband")
            tmp = cpool.tile([P14, NIH * W], dt, tag="tmp")
            cselS = cpool.tile([NSHORT, NI], dt, tag="cselS")
            cselL = cpool.tile([NLONG, NI], dt, tag="cselL")
            pslott = cpool.tile([2, NSLOT], dt, tag="pslot")
            pslott2 = cpool.tile([NIH, NSLOT], dt, tag="pslot2")
            identt = cpool.tile([P14, 64], dt, tag="ident")
            # ---- init
            nc.vector.memset(lampad[:, :], float(NEG))
            nc.sync.dma_start(fhsd[:, :, :], fhsinit[:, :, :])
            nc.sync.dma_start(tbandt[:, :], tb_in[:, :])
            nc.sync.dma_start(cselS[:, :], cs_in[0:NSHORT, :])
            nc.sync.dma_start(cselL[:, :], cs_in[NSHORT:NSLOT, :])
            nc.sync.dma_start(pslott[:, :], ps_in[0:2, :])
            nc.sync.dma_start(pslott2[:, :], ps_in[2:32, :])
            nc.sync.dma_start(identt[:, :], id_in[:, :])
            dws = psp.tile([128, 128], dt, tag="dwarm")
            nc.tensor.matmul(dws[0:NI, 0:NI], cselS[:, :], cselS[:, :],
                             start=True, stop=True, skip_group_check=True)
            nc.tensor.matmul(dws[0:NI, 0:NI], cselL[:, :], cselL[:, :],
                             start=True, stop=True, skip_group_check=True)
            nc.tensor.matmul(dws[0:NSLOT, 0:NSLOT], pslott[:, :],
                             pslott[:, :], start=True, stop=True,
                             skip_group_check=True)
            nc.tensor.matmul(dws[0:NSLOT, 0:NSLOT], pslott2[:, :],
                             pslott2[:, :], start=True, stop=True,
                             skip_group_check=True)
            nc.tensor.matmul(dws[0:64, 0:64], identt[0:64, :],
                             identt[0:64, :], start=True, stop=True,
                             skip_group_check=True)

            lazy = None
            lazy2 = []
            for k in range(NBLK14):
                t0 = k * TAU14
                # ---- window loads
                winL = pool.tile([NLONG, 64], dt, tag="winL")
                win_load(nc.sync.dma_start, nc.scalar.dma_start,
                         winL, NSHORT, NLONG, t0)
                winS = pool.tile([NSHORT, 64], dt, tag="winS")
                win_load(nc.sync.dma_start, nc.scalar.dma_start,
                         winS, 0, NSHORT, t0)
                # ---- T1: 4 matmuls (h x {long, short}) -> pst [128, 60]
                psth = []
                for h in range(H14):
                    ph = psp.tile([64, NI], dt, tag=f"lamraw{h}")
                    psth.append(ph)
                for h in range(H14):
                    lhsL = winL[:, :].copy()
                    lhsL.ap = mybir.VecI64Pair([[64, NLONG], [1, 64]])
                    lhsL.offset = 0
                    nc.tensor.matmul(psth[h][:, :], lhsL, cselL[:, :],
                                     start=True, stop=False)
                for h in range(H14):
                    lhsS = winS[:, :].copy()
                    lhsS.ap = mybir.VecI64Pair([[64, NSHORT], [1, 64]])
                    lhsS.offset = 0
                    nc.tensor.matmul(psth[h][:, :], lhsS, cselS[:, :],
                                     start=False, stop=True)
                # ---- deferred history writes (emitted late so conservative
                # DRAM dep tracking orders this block's reads first: WAR).
                # D1b deferred 1 block, D1z deferred 2 blocks.
                if lazy is not None:
                    lz_s60a, lz_stg, lz_t0 = lazy
                    nc.sync.dma_start(fhs_diag(0, NSHORT, lz_t0),
                                      stg_src(lz_s60a, 0, NSHORT))
                    lazy2.append((lz_stg, lz_t0))
                    lazy = None
                if len(lazy2) > 1:
                    lz_stg, lz_t0 = lazy2.pop(0)
                    nc.gpsimd.dma_start(fhs_diag(NSHORT, NLONG, lz_t0),
                                        stg_src(lz_stg, NSHORT, NLONG))
                # ---- V1 (two halves; h1 columns shifted by -30)
                sct = pool.tile([P14, NI], dt, tag="sc")
                nc.gpsimd.dma_start(sct[:, :], sc_in[k, :, :])
                nc.vector.tensor_add(out=lampad[0:64, 15:LPW],
                                     in0=psth[0][:, 0:LPW - 15],
                                     in1=sct[0:64, 0:LPW - 15])
                nc.vector.tensor_add(out=lampad[64:P14, 0:45],
                                     in0=psth[1][:, 15:NI],
                                     in1=sct[64:P14, 15:NI])
                # ---- V2 (single op over all rows)
                lam_win = lampad[:, :].copy()
                lam_win.ap = mybir.VecI64Pair([[LPW, P14], [1, NIH], [1, W]])
                lam_win.offset = 0
                nc.vector.tensor_add(out=tmp[:, :], in0=lam_win,
                                     in1=tbandt[:, :])
                # ---- V3 (single reduce)
                tmp3 = tmp[:, :].copy()
                tmp3.ap = mybir.VecI64Pair([[NIH * W, P14], [W, NIH], [1, W]])
                fk = pool.tile([P14, NIH], dt, tag="fk")
                nc.vector.tensor_reduce(out=fk[:, :], in_=tmp3,
                                        axis=mybir.AxisListType.X,
                                        op=mybir.AluOpType.max)
                nc.gpsimd.dma_start(fout[k, :, :], fk[:, :])
                # ---- short tail: transpose j<28 (h0 rows), stage, D1b
                pst2a = psp.tile([28, 64], dt, tag="ftra")
                nc.tensor.transpose(pst2a[:, :], fk[0:64, 0:28],
                                    identt[0:64, :])
                s60a = pool.tile([28, 64], dt, tag="s60a")
                nc.scalar.copy(out=s60a[:, :], in_=pst2a[:, :])
                # ---- lazy path: long j (h0 cols 28:30, h1 all) -> slot-pad
                pst2b1 = psp.tile([2, 64], dt, tag="ftrb1")
                nc.tensor.transpose(pst2b1[:, :], fk[0:64, 28:NIH],
                                    identt[0:64, :])
                pst2b2 = psp.tile([NIH, 64], dt, tag="ftrb2")
                nc.tensor.transpose(pst2b2[:, :], fk[64:P14, 0:NIH],
                                    identt[64:P14, :])
                s60b1 = pool.tile([2, 64], dt, tag="s60b1")
                nc.scalar.copy(out=s60b1[:, :], in_=pst2b1[:, :])
                s60b2 = pool.tile([NIH, 64], dt, tag="s60b2")
                nc.scalar.copy(out=s60b2[:, :], in_=pst2b2[:, :])
                pst3 = psp.tile([NSLOT, 64], dt, tag="slotp")
                nc.tensor.matmul(pst3[:, :], pslott[0:2, :], s60b1[:, :],
                                 start=True, stop=False)
                nc.tensor.matmul(pst3[:, :], pslott2[:, :], s60b2[:, :],
                                 start=False, stop=True)
                stg = pool.tile([NSLOT, 64], dt, tag="stg")
                nc.scalar.copy(out=stg[:, :], in_=pst3[:, :])
                lazy = (s60a, stg, t0)
            if lazy is not None:
                lz_s60a, lz_stg, lz_t0 = lazy
                nc.sync.dma_start(fhs_diag(0, NSHORT, lz_t0),
                                  stg_src(lz_s60a, 0, NSHORT))
    nc.compile()
    return nc


USE14 = False


def _run_device(Sc, pre_full, tband, csel, pslot, ident, dens, iv, T,
                trace=False):
    from concourse.bass_utils import run_bass_kernel_spmd

    if USE14:
        Sc14, tb14, ident64 = _dev14_arrays(dens, Sc, tband, iv, T)
        nc = _build_bass14(Sc14, tb14, csel, pslot, ident64)
        in_map = {"sc": np.ascontiguousarray(Sc14),
                  "fhsinit": np.ascontiguousarray(pre_full),
                  "tb": np.ascontiguousarray(tb14),
                  "cs": np.ascontiguousarray(csel),
                  "ps": np.ascontiguousarray(pslot),
                  "id": np.ascontiguousarray(ident64)}
    else:
        nc = _build_bass(None, Sc, pre_full, tband, csel, pslot, ident)
        in_map = {"sc": np.ascontiguousarray(Sc),
                  "fhsinit": np.ascontiguousarray(pre_full),
                  "tb": np.ascontiguousarray(tband),
                  "cs": np.ascontiguousarray(csel),
                  "ps": np.ascontiguousarray(pslot),
                  "id": np.ascontiguousarray(ident)}
    core_ids = list(range(8))
    res = run_bass_kernel_spmd(nc, [dict(in_map) for _ in core_ids], core_ids,
                               trace=trace)
    return res.results[0]["fout"], res


# ------------------------------------------------------------- host finalize
def _finalize(fh, dens, Sc, iv, T, num_states, maxiv):
    """Final v over all states, argmax, logp, backtrack."""
    S = int(num_states)
    iv_cum = np.concatenate([[0], np.cumsum(iv)]).astype(np.int64)
    nsb = int(iv_cum[-1])
    v_fin = np.full(S, NEG, np.float32)
    for b in range(NB):
        beta = 2 if b == 0 else 1
        for j in range(NI):
            L = int(iv[j])
            thr = math.ceil(L / OBS_LAMBDA)
            fs = b * nsb + int(iv_cum[j])
            ks = np.arange(L)
            classes = np.where(ks < thr, beta, 0)
            t0s = F - 1 - ks
            tails = np.zeros(L, np.float32)
            for kk in range(1, L):
                xs = np.arange(1, kk + 1)
                tails[kk] = dens[t0s[kk] + xs, classes[xs]].astype(
                    np.float32).sum()
            v_fin[fs:fs + L] = (fh[t0s + maxiv, b, j].astype(np.float32)
                                + tails)
    last_state = int(np.argmax(v_fin))
    logp = np.float32(v_fin[last_state])

    Tf = T.astype(np.float32)
    path = np.zeros(F, np.int32)
    path[F - 1] = last_state
    s = last_state
    jsr = np.arange(NI)
    for t in range(F - 1, 0, -1):
        b = s // nsb
        r = s % nsb
        j = int(np.searchsorted(iv_cum, r, side="right")) - 1
        if r - iv_cum[j] > 0:
            s = s - 1
        else:
            bp = (b - 1) % NB
            k, tau = divmod(t, TAU)
            fd = fh[t - iv + maxiv, bp, jsr].astype(np.float32)
            cand = np.float32(np.float32(fd + Sc[k, b * BSTR + tau, :])
                              + Tf[:, j])
            # restrict to band (device computes only these)
            mask = (jsr - j >= BLO) & (jsr - j <= BHI)
            cand = np.where(mask, cand, NEG)
            jstar = int(np.argmax(cand))
            s = int(bp * nsb + iv_cum[jstar + 1] - 1)
        path[t - 1] = s
    return path, logp


# ------------------------------------------------------------------- kernel
def kernel(activations, log_trans, prev_last, first_states, pointer,
           num_states, backend="device", trace=False, _ret_extra=False):
    iv, slot, T = _statics(first_states, log_trans)
    (dens, Sc, fvirt, maxiv, pre, tband, csel, pslot, ident) = _precompute(
        activations, iv, slot, T, num_states)

    if backend == "numpy":
        _forward_numpy.tband_row = np.asarray(
            tband[0].reshape(NI, W))
        fh = _forward_numpy(Sc, fvirt, maxiv, iv, T)
        res = None
    else:
        fout, res = _run_device(Sc, pre, tband, csel, pslot, ident,
                                dens, iv, T, trace=trace)
        fh = np.empty((F + maxiv, NB, NI), np.float32)
        fh[:maxiv] = np.moveaxis(fvirt, 2, 0).astype(np.float32)
        if USE14:
            f4 = np.asarray(fout).reshape(NBLK14, H14, NB, BST14, NIH)
            for t in range(F):
                k, tau = divmod(t, TAU14)
                for b in range(NB):
                    fh[maxiv + t, b, 0:NIH] = f4[k, 0, b, 2 + tau, :]
                    fh[maxiv + t, b, NIH:NI] = f4[k, 1, b, 2 + tau, :]
        else:
            f3 = np.asarray(fout).reshape(NBLK, NB, BSTR, NI)[:, :, :TAU]
            fflat = np.moveaxis(f3, 2, 1).reshape(NBLK * TAU, NB, NI)
            fh[maxiv:] = fflat[:F]

    path, logp = _finalize(fh, dens, Sc, iv, T, num_states, maxiv)
    if _ret_extra:
        return (path, logp), res, fh
    return path, logp


def timeline_estimate(inputs):
    """Cost-model (TimelineSim) estimate of single-core kernel exec ns."""
    from concourse.timeline_sim import TimelineSim
    iv, slot, T = _statics(inputs["first_states"], inputs["log_trans"])
    (dens, Sc, fvirt, maxiv, pre, tband, csel, pslot, ident) = _precompute(
        inputs["activations"], iv, slot, T, inputs["num_states"])
    if USE14:
        Sc14, tb14, ident64 = _dev14_arrays(dens, Sc, tband, iv, T)
        nc = _build_bass14(Sc14, tb14, csel, pslot, ident64)
    else:
        nc = _build_bass(None, Sc, pre, tband, csel, pslot, ident)
    return int(TimelineSim(nc, trace=False).simulate())
